# revision 14
# baseline (speedup 1.0000x reference)
"""Trainium2 Bass kernel for nn_NeuralPolarDecoder.

Data-parallel over 8 NeuronCores (batch 256 -> 32/core). Per core, the
polar-decoder stage recursion runs with features on SBUF partitions and
(sample, pair) on the free axis:

  RHS_s [128, 16384]: rows 0:64 = odd-operand features, 64:128 = even-operand.
  stage: h = relu(W1cat.T @ RHS + vxor-rank1 + b1)   (one K=128 matmul + K=1)
         e' = W2blk.T @ h  (block-diag cn/bn)  -> psum [e'L ; e'R]
         ES = e' + b2cat   (SBUF scratch)
         d  = wd.T @ ES  (pred head), nsq = ones.T @ ES^2 -- via shifted
         weights, 8 chunks accumulate into one psum bank at rows 0:16/64:80
         RHS_{s+1} built from ES by 4 quadrant moves (2 strided engine copies,
         2 partition-crossing DMAs).
  Final batch: softplus/sigmoid/sqrt over stacked per-stage d/nsq planes
  (round-tripped through internal DRAM to fit SBUF).

Label-bit (v) evolution and all weight folding are precomputed on host;
device outputs are packed [128, 11*256] tiles unpacked on host.
"""

import numpy as np

B, N, D = 256, 1024, 64
NCORES = 8
BLOC = B // NCORES          # 32 samples per core
T = BLOC * N                # 32768 tokens per core
P = T // 2                  # 16384 pairs per core
S = 11                      # stages (incl. stage 0)
LOSS_CAP = 16.11809565095832

_COMPILED = {}


def _host_prep(y, x, w):
    """Per-core host packing. y (32,1024,2) f32, x (32,1024,1) i32."""
    b = y.shape[0]
    yf = y.reshape(b * N, 2)
    y_rhs = np.concatenate([yf[0::2].T, yf[1::2].T], axis=0).astype(np.float32)  # [4,P]

    vflat = x.reshape(b * N).astype(np.float32)
    Vo = vflat[0::2].copy()
    Ve = vflat[1::2].copy()
    vxor_rows = np.zeros((10, P), np.float32)
    sign_in = np.zeros((128, S * 256), np.float32)

    def pack_sign(s, vL, vR):
        # value (r,bb,q) -> [p = (q//256)*64 + 2bb + r, col s*256 + q%256]
        for r, v in ((0, vL), (1, vR)):
            vv = v.reshape(b, 512)
            for qh in range(2):
                blk = 1.0 - 2.0 * vv[:, qh * 256:(qh + 1) * 256]
                sign_in[qh * 64 + 2 * np.arange(b) + r, s * 256:(s + 1) * 256] = blk

    pack_sign(0, Vo, Ve)
    q = np.arange(512)
    for s in range(1, 11):
        vx = (Vo - Ve) ** 2
        vxor_rows[s - 1] = vx
        pack_sign(s, vx, Ve)
        if s < 10:
            bit = (q >> (s - 1)) & 1
            qdel = ((q >> s) << (s - 1)) | (q & ((1 << (s - 1)) - 1))
            m0 = bit == 0
            jp0 = 2 * qdel[m0]
            jp1 = 2 * qdel[~m0]
            Vor = Vo.reshape(b, 512); Ver = Ve.reshape(b, 512)
            vxr = vx.reshape(b, 512)
            nVo = np.empty_like(Vor); nVe = np.empty_like(Ver)
            nVo[:, jp0] = vxr[:, q[m0]]; nVo[:, jp0 + 1] = Ver[:, q[m0]]
            nVe[:, jp1] = vxr[:, q[~m0]]; nVe[:, jp1 + 1] = Ver[:, q[~m0]]
            Vo, Ve = nVo.reshape(P), nVe.reshape(P)

    return {"y_rhs": y_rhs, "vxor": vxor_rows, "sign_in": sign_in, **w}


def _fold_weights(inp):
    f = np.float32
    cn_W1, cn_b1 = inp["cn_W1"].astype(f), inp["cn_b1"].astype(f)
    cn_W2, cn_b2 = inp["cn_W2"].astype(f), inp["cn_b2"].astype(f)
    bn_W1, bn_b1 = inp["bn_W1"].astype(f), inp["bn_b1"].astype(f)
    bn_W2, bn_b2 = inp["bn_W2"].astype(f), inp["bn_b2"].astype(f)
    emb_W, emb_b = inp["emb_W"].astype(f), inp["emb_b"].astype(f)
    lab_emb = inp["lab_emb"].astype(f)
    llr_W, llr_b = inp["llr_W"].astype(f), inp["llr_b"].astype(f)

    W1cat = np.concatenate([cn_W1, bn_W1[:128]], axis=1)            # [128,128]
    r0 = lab_emb[0] @ bn_W1[128:192]
    r1 = lab_emb[1] @ bn_W1[128:192]
    rank1 = np.concatenate([np.zeros(64, f), (r1 - r0).astype(f)])[None, :]  # [1,128]
    b1_eff = np.concatenate([cn_b1, bn_b1 + r0])[:, None].astype(f)
    W2blk = np.zeros((128, 128), f)
    W2blk[:64, :64] = cn_W2; W2blk[64:, 64:] = bn_W2
    b2cat = np.concatenate([cn_b2, bn_b2])[:, None].astype(f)
    w_d = (llr_W[:, 1] - llr_W[:, 0]).astype(f)
    delta = float(llr_b[1] - llr_b[0])
    # shifted weights: block cc has the head at cols 2cc:2cc+2 of [128,16]
    wd_sh = np.zeros((128, 8, 128), f)
    ones_sh = np.zeros((128, 8, 128), f)
    for cc in range(8):
        wd_sh[:64, cc, 2 * cc] = w_d
        wd_sh[64:, cc, 2 * cc + 1] = w_d
        ones_sh[:64, cc, 64 + 2 * cc] = 1
        ones_sh[64:, cc, 64 + 2 * cc + 1] = 1
    wd_sh = wd_sh.reshape(128, 1024)
    ones_sh = ones_sh.reshape(128, 1024)
    W0cat2 = np.zeros((4, 128), f)
    W0cat2[:2, :64] = emb_W; W0cat2[2:, 64:] = emb_W
    embb_cat = np.concatenate([emb_b, emb_b])[:, None].astype(f)
    dbias_sh = np.zeros((128, 1), f)
    dbias_sh[:16] = delta
    return {
        "W1cat": W1cat, "rank1": rank1, "b1_eff": b1_eff, "W2blk": W2blk,
        "b2cat": b2cat, "wd_sh": wd_sh, "ones_sh": ones_sh,
        "W0cat2": W0cat2, "embb_cat": embb_cat, "dbias_sh": dbias_sh,
    }


def _build_nc():
    import concourse.bacc as bacc
    import concourse.mybir as mybir
    import concourse.tile as tile
    from contextlib import ExitStack

    dt = mybir.dt
    f32, f32r = dt.float32, dt.float32r
    AF = mybir.ActivationFunctionType

    nc = bacc.Bacc("TRN2", target_bir_lowering=False, debug=False,
                   num_devices=NCORES)

    din = {}
    MM_INPUTS = {"y_rhs", "vxor", "W1cat", "rank1", "W2blk", "wd_sh",
                 "ones_sh", "W0cat2"}
    for name, shape in [
        ("y_rhs", [4, P]), ("vxor", [10, P]), ("sign_in", [128, S * 256]),
        ("W1cat", [128, 128]), ("rank1", [1, 128]), ("b1_eff", [128, 1]),
        ("W2blk", [128, 128]), ("b2cat", [128, 1]), ("wd_sh", [128, 1024]),
        ("ones_sh", [128, 1024]), ("W0cat2", [4, 128]), ("embb_cat", [128, 1]),
        ("dbias_sh", [128, 1]),
    ]:
        dtt = f32r if name in MM_INPUTS else f32
        din[name] = nc.dram_tensor(name, shape, dtt, kind="ExternalInput").ap()
    dout = {}
    for name in ["loss_o", "p0_o", "p1_o", "nrm_o"]:
        dout[name] = nc.dram_tensor(name, [128, S * 256], f32,
                                    kind="ExternalOutput").ap()
    dall_d = nc.dram_tensor("dall_i", [128, S * 256], f32).ap()
    nall_d = nc.dram_tensor("nall_i", [128, S * 256], f32).ap()

    with tile.TileContext(nc) as tc, ExitStack() as ctx:
        wp = ctx.enter_context(tc.tile_pool(name="w", bufs=1))
        big = ctx.enter_context(tc.tile_pool(name="big", bufs=1))
        yp = ctx.enter_context(tc.tile_pool(name="yp", bufs=3))
        vp = ctx.enter_context(tc.tile_pool(name="vp", bufs=3))
        hp = ctx.enter_context(tc.tile_pool(name="hp", bufs=3))
        sqp = ctx.enter_context(tc.tile_pool(name="sqp", bufs=2))
        esp = ctx.enter_context(tc.tile_pool(name="esp", bufs=2))
        smp = ctx.enter_context(tc.tile_pool(name="smp", bufs=2))
        scp = ctx.enter_context(tc.tile_pool(name="scp", bufs=2))
        fpool = ctx.enter_context(tc.tile_pool(name="fp", bufs=1))
        ph = ctx.enter_context(tc.tile_pool(name="ph", bufs=2, space="PSUM"))
        pe = ctx.enter_context(tc.tile_pool(name="pe", bufs=2, space="PSUM"))
        psm = ctx.enter_context(tc.tile_pool(name="psm", bufs=2, space="PSUM"))

        wt = {}
        for name in ["W1cat", "rank1", "b1_eff", "W2blk", "b2cat", "wd_sh",
                     "ones_sh", "W0cat2", "embb_cat", "dbias_sh"]:
            shape = list(din[name].shape)
            wt[name] = wp.tile(shape, din[name].dtype, name=name, tag=name)
            nc.sync.dma_start(wt[name][:], din[name][:])

        RHS_A = big.tile([128, P], f32r)
        RHS_B = big.tile([128, P], f32r)

        def smalls(psm_t, esv, c):
            cc = c % 8
            nc.tensor.matmul(psm_t[:, :],
                             wt["wd_sh"][:, 128 * cc:128 * (cc + 1)],
                             esv,
                             start=(cc == 0), stop=False,
                             skip_group_check=True)
            sq_t = sqp.tile([128, 512], f32r)
            if c % 2 == 0:
                nc.scalar.activation(sq_t[:], esv, AF.Square)
            else:
                nc.vector.tensor_mul(sq_t[:], esv, esv)
            nc.tensor.matmul(psm_t[:, :],
                             wt["ones_sh"][:, 128 * cc:128 * (cc + 1)],
                             sq_t[:],
                             start=False, stop=(cc == 7),
                             skip_group_check=True)

        def smalls_out(s, k, psm_t):
            sm_t = smp.tile([128, 512], f32)
            nc.scalar.activation(sm_t[:], psm_t[:], AF.Identity,
                                 bias=wt["dbias_sh"][:])
            for half, dst in ((0, dall_d), (64, nall_d)):
                src = sm_t[half:half + 16, :].rearrange(
                    "p (qh ql) -> p qh ql", qh=2, ql=256)
                dv = dst.rearrange("(qh k2 ccr) c -> k2 ccr qh c",
                                   qh=2, k2=4, ccr=16)
                nc.sync.dma_start(
                    out=dv[k, :, :, s * 256:(s + 1) * 256], in_=src)

        def q_moves(s, es_t, nxt, g2):
            ql = 1 << (s - 1)
            bqh = 1024 // (2 * ql)
            sc_t = scp.tile([128, 1024], f32r)
            # contiguous partition shifts (DMA can't do strided scatter)
            nc.sync.dma_start(sc_t[0:64, :], es_t[64:128, :])
            nc.sync.dma_start(sc_t[64:128, :], es_t[0:64, :])
            esr = es_t[:].rearrange("p (bqh rest) -> p bqh rest", bqh=bqh)
            scr = sc_t[:].rearrange("p (bqh rest) -> p bqh rest", bqh=bqh)
            dsl = nxt[:, g2 * 1024:(g2 + 1) * 1024].rearrange(
                "p (bqh ql2 r2) -> p bqh ql2 r2", bqh=bqh, ql2=ql, r2=2)
            dsq = dsl[:, :, :, 0]
            dsr = dsl[:, :, :, 1]
            nc.scalar.copy(dsq[0:64], esr[0:64, :, 0:ql])          # Q1
            nc.vector.tensor_copy(dsr[64:128], esr[64:128, :, ql:2 * ql])  # Q4
            nc.vector.tensor_copy(dsr[0:64], scr[0:64, :, 0:ql])   # Q2
            nc.scalar.copy(dsq[64:128], scr[64:128, :, ql:2 * ql])  # Q3

        # ---------------- stage 0 ----------------
        for k in range(4):
            psm_t = psm.tile([128, 512], f32)
            for cc in range(8):
                c = 8 * k + cc
                yt = yp.tile([4, 512], f32r)
                nc.sync.dma_start(yt[:], din["y_rhs"][:, c * 512:(c + 1) * 512])
                ph_t = ph.tile([128, 512], f32)
                nc.tensor.matmul(ph_t[:], wt["W0cat2"][:],
                                 yt[:], start=True, stop=True)
                esv = RHS_A[:, c * 512:(c + 1) * 512]
                nc.vector.tensor_scalar_add(esv, ph_t[:], wt["embb_cat"][:])
                smalls(psm_t, esv, c)
            smalls_out(0, k, psm_t)

        # ---------------- stages 1..10 ----------------
        for s in range(1, 11):
            cur = RHS_A if s % 2 == 1 else RHS_B
            nxt = RHS_B if s % 2 == 1 else RHS_A
            for k in range(4):
                psm_t = psm.tile([128, 512], f32)
                for cc in range(8):
                    c = 8 * k + cc
                    if cc % 2 == 0:
                        es_t = esp.tile([128, 1024], f32r)
                    vx_t = vp.tile([1, 512], f32r)
                    nc.sync.dma_start(
                        vx_t[:], din["vxor"][s - 1:s, c * 512:(c + 1) * 512])
                    ph_t = ph.tile([128, 512], f32)
                    nc.tensor.matmul(ph_t[:], wt["W1cat"][:],
                                     cur[:, c * 512:(c + 1) * 512],
                                     start=True, stop=False,
                                     skip_group_check=True)
                    nc.tensor.matmul(ph_t[:, :],
                                     wt["rank1"][:],
                                     vx_t[:],
                                     start=False, stop=True,
                                     skip_group_check=True)
                    h_t = hp.tile([128, 512], f32r)
                    nc.scalar.activation(h_t[:], ph_t[:], AF.Relu,
                                         bias=wt["b1_eff"][:])
                    pe_t = pe.tile([128, 512], f32)
                    nc.tensor.matmul(pe_t[:], wt["W2blk"][:],
                                     h_t[:], start=True,
                                     stop=True)
                    esv = es_t[:, (cc % 2) * 512:(cc % 2) * 512 + 512]
                    nc.vector.tensor_scalar_add(esv, pe_t[:], wt["b2cat"][:])
                    smalls(psm_t, esv, c)
                    if s < 10 and cc % 2 == 1:
                        q_moves(s, es_t, nxt, c // 2)
                smalls_out(s, k, psm_t)

        # ---------------- final ----------------
        FW = S * 256
        NSL = 8
        SL = FW // NSL                     # 352
        for i in range(NSL):               # loss pass (softplus)
            csl = slice(i * SL, (i + 1) * SL)
            dq = fpool.tile([128, SL], f32, tag="dq")
            nc.sync.dma_start(dq[:], dall_d[:, csl])
            sq = fpool.tile([128, SL], f32, tag="sq")
            nc.sync.dma_start(sq[:], din["sign_in"][:, csl])
            zn = fpool.tile([128, SL], f32, tag="zn")
            nc.vector.tensor_mul(zn[:], sq[:], dq[:])
            ex = fpool.tile([128, SL], f32, tag="ex")
            nc.scalar.activation(ex[:], zn[:], AF.Exp)
            e1 = fpool.tile([128, SL], f32, tag="e1")
            nc.vector.tensor_scalar_add(e1[:], ex[:], 1.0)
            lp = fpool.tile([128, SL], f32, tag="lp")
            nc.scalar.activation(lp[:], e1[:], AF.Ln)
            lo = fpool.tile([128, SL], f32, tag="lo")
            nc.vector.tensor_scalar_min(lo[:], lp[:], LOSS_CAP)
            nc.sync.dma_start(dout["loss_o"][:, csl], lo[:])
        for i in range(NSL):               # pred pass (sigmoid)
            csl = slice(i * SL, (i + 1) * SL)
            dq = fpool.tile([128, SL], f32, tag="dq")
            nc.sync.dma_start(dq[:], dall_d[:, csl])
            p1t = fpool.tile([128, SL], f32, tag="p1")
            nc.scalar.activation(p1t[:], dq[:], AF.Sigmoid)
            nc.sync.dma_start(dout["p1_o"][:, csl], p1t[:])
            p0t = fpool.tile([128, SL], f32, tag="p0")
            nc.scalar.activation(p0t[:], dq[:], AF.Sigmoid, scale=-1.0)
            nc.sync.dma_start(dout["p0_o"][:, csl], p0t[:])
        for i in range(NSL):               # norm pass (sqrt)
            csl = slice(i * SL, (i + 1) * SL)
            nq = fpool.tile([128, SL], f32, tag="nq")
            nc.sync.dma_start(nq[:], nall_d[:, csl])
            nrt = fpool.tile([128, SL], f32, tag="nr")
            nc.scalar.activation(nrt[:], nq[:], AF.Sqrt)
            nc.sync.dma_start(dout["nrm_o"][:, csl], nrt[:])

    nc.compile()
    return nc


def _unpack(core_outs):
    """core_outs: dict of [128, S*256] arrays -> (losses, preds, norms)."""
    b = BLOC

    def unpk(a):
        a = a.reshape(2, 64, S, 256)            # (qh, 2b+r, s, ql)
        a = a.reshape(2, b, 2, S, 256)          # (qh, bb, r, s, ql)
        a = a.transpose(3, 2, 1, 0, 4).reshape(S, 2, b, 512)
        return a                                # [s, r, b, q]

    lo = unpk(core_outs["loss_o"]); p0 = unpk(core_outs["p0_o"])
    p1 = unpk(core_outs["p1_o"]); nr = unpk(core_outs["nrm_o"])
    losses = np.zeros((b, S, N), np.float32)
    preds = np.zeros((b, N, S, 2), np.float32)
    norms = np.zeros((b, S, N), np.float32)
    for r in range(2):
        losses[:, :, r::2] = lo[:, r].transpose(1, 0, 2)
        norms[:, :, r::2] = nr[:, r].transpose(1, 0, 2)
        preds[:, r::2, :, 0] = p0[:, r].transpose(1, 2, 0)
        preds[:, r::2, :, 1] = p1[:, r].transpose(1, 2, 0)
    return losses, preds, norms


def kernel(x, y, emb_W, emb_b, lab_emb, cn_W1, cn_b1, cn_W2, cn_b2,
           bn_W1, bn_b1, bn_W2, bn_b2, llr_W, llr_b, _trace=False):
    from concourse.bass_utils import run_bass_kernel_spmd

    inp = dict(x=np.asarray(x), y=np.asarray(y))
    w = _fold_weights(dict(
        emb_W=np.asarray(emb_W), emb_b=np.asarray(emb_b),
        lab_emb=np.asarray(lab_emb), cn_W1=np.asarray(cn_W1),
        cn_b1=np.asarray(cn_b1), cn_W2=np.asarray(cn_W2),
        cn_b2=np.asarray(cn_b2), bn_W1=np.asarray(bn_W1),
        bn_b1=np.asarray(bn_b1), bn_W2=np.asarray(bn_W2),
        bn_b2=np.asarray(bn_b2), llr_W=np.asarray(llr_W),
        llr_b=np.asarray(llr_b)))

    if "nc" not in _COMPILED:
        _COMPILED["nc"] = _build_nc()
    nc = _COMPILED["nc"]

    in_maps = []
    for ci in range(NCORES):
        sl = slice(ci * BLOC, (ci + 1) * BLOC)
        in_maps.append(_host_prep(inp["y"][sl], inp["x"][sl], w))

    res = run_bass_kernel_spmd(nc, in_maps, list(range(NCORES)),
                               trace=_trace)
    losses = np.zeros((B, S, N), np.float32)
    preds = np.zeros((B, N, S, 2), np.float32)
    norms = np.zeros((B, S, N), np.float32)
    for ci in range(NCORES):
        lo, pr, no = _unpack(res.results[ci])
        sl = slice(ci * BLOC, (ci + 1) * BLOC)
        losses[sl], preds[sl], norms[sl] = lo, pr, no
    kernel._last = res
    return losses, preds, norms


# revision 15
# speedup vs baseline: 1.1740x; 1.1740x over previous
"""Trainium2 Bass kernel for nn_NeuralPolarDecoder.

Data-parallel over 8 NeuronCores (batch 256 -> 32/core). Per core, the
polar-decoder stage recursion runs with features on SBUF partitions and
(sample, pair) on the free axis:

  RHS_s [128, 16384]: rows 0:64 = odd-operand features, 64:128 = even-operand.
  stage: h = relu(W1cat.T @ RHS + vxor-rank1 + b1)   (one K=128 matmul + K=1)
         e' = W2blk.T @ h  (block-diag cn/bn)  -> psum [e'L ; e'R]
         ES = e' + b2cat   (SBUF scratch)
         d  = wd.T @ ES  (pred head), nsq = ones.T @ ES^2 -- via shifted
         weights, 8 chunks accumulate into one psum bank at rows 0:16/64:80
         RHS_{s+1} built from ES by 4 quadrant moves (2 strided engine copies,
         2 partition-crossing DMAs).
  Final batch: softplus/sigmoid/sqrt over stacked per-stage d/nsq planes
  (round-tripped through internal DRAM to fit SBUF).

Label-bit (v) evolution and all weight folding are precomputed on host;
device outputs are packed [128, 11*256] tiles unpacked on host.
"""

import numpy as np

B, N, D = 256, 1024, 64
NCORES = 8
BLOC = B // NCORES          # 32 samples per core
T = BLOC * N                # 32768 tokens per core
P = T // 2                  # 16384 pairs per core
S = 11                      # stages (incl. stage 0)
LOSS_CAP = 16.11809565095832

_COMPILED = {}


def _host_prep(y, x, w):
    """Per-core host packing. y (32,1024,2) f32, x (32,1024,1) i32."""
    b = y.shape[0]
    yf = y.reshape(b * N, 2)
    y_rhs = np.concatenate([yf[0::2].T, yf[1::2].T], axis=0).astype(np.float32)  # [4,P]

    vflat = x.reshape(b * N).astype(np.float32)
    Vo = vflat[0::2].copy()
    Ve = vflat[1::2].copy()
    vxor_rows = np.zeros((10, P), np.float32)
    sign_in = np.zeros((128, S * 256), np.float32)

    def pack_sign(s, vL, vR):
        # value (r,bb,q) -> [p = (q//256)*64 + 2bb + r, col s*256 + q%256]
        for r, v in ((0, vL), (1, vR)):
            vv = v.reshape(b, 512)
            for qh in range(2):
                blk = 1.0 - 2.0 * vv[:, qh * 256:(qh + 1) * 256]
                sign_in[qh * 64 + 2 * np.arange(b) + r, s * 256:(s + 1) * 256] = blk

    pack_sign(0, Vo, Ve)
    q = np.arange(512)
    for s in range(1, 11):
        vx = (Vo - Ve) ** 2
        vxor_rows[s - 1] = vx
        pack_sign(s, vx, Ve)
        if s < 10:
            bit = (q >> (s - 1)) & 1
            qdel = ((q >> s) << (s - 1)) | (q & ((1 << (s - 1)) - 1))
            m0 = bit == 0
            jp0 = 2 * qdel[m0]
            jp1 = 2 * qdel[~m0]
            Vor = Vo.reshape(b, 512); Ver = Ve.reshape(b, 512)
            vxr = vx.reshape(b, 512)
            nVo = np.empty_like(Vor); nVe = np.empty_like(Ver)
            nVo[:, jp0] = vxr[:, q[m0]]; nVo[:, jp0 + 1] = Ver[:, q[m0]]
            nVe[:, jp1] = vxr[:, q[~m0]]; nVe[:, jp1 + 1] = Ver[:, q[~m0]]
            Vo, Ve = nVo.reshape(P), nVe.reshape(P)

    return {"y_rhs": y_rhs, "vxor": vxor_rows, "sign_in": sign_in, **w}


def _fold_weights(inp):
    f = np.float32
    cn_W1, cn_b1 = inp["cn_W1"].astype(f), inp["cn_b1"].astype(f)
    cn_W2, cn_b2 = inp["cn_W2"].astype(f), inp["cn_b2"].astype(f)
    bn_W1, bn_b1 = inp["bn_W1"].astype(f), inp["bn_b1"].astype(f)
    bn_W2, bn_b2 = inp["bn_W2"].astype(f), inp["bn_b2"].astype(f)
    emb_W, emb_b = inp["emb_W"].astype(f), inp["emb_b"].astype(f)
    lab_emb = inp["lab_emb"].astype(f)
    llr_W, llr_b = inp["llr_W"].astype(f), inp["llr_b"].astype(f)

    W1cat = np.concatenate([cn_W1, bn_W1[:128]], axis=1)            # [128,128]
    r0 = lab_emb[0] @ bn_W1[128:192]
    r1 = lab_emb[1] @ bn_W1[128:192]
    rank1 = np.concatenate([np.zeros(64, f), (r1 - r0).astype(f)])[None, :]  # [1,128]
    b1_eff = np.concatenate([cn_b1, bn_b1 + r0])[:, None].astype(f)
    W2blk = np.zeros((128, 128), f)
    W2blk[:64, :64] = cn_W2; W2blk[64:, 64:] = bn_W2
    b2cat = np.concatenate([cn_b2, bn_b2])[:, None].astype(f)
    w_d = (llr_W[:, 1] - llr_W[:, 0]).astype(f)
    delta = float(llr_b[1] - llr_b[0])
    # shifted weights: block cc has the head at cols 2cc:2cc+2 of [128,16]
    wd_sh = np.zeros((128, 8, 128), f)
    ones_sh = np.zeros((128, 8, 128), f)
    for cc in range(8):
        wd_sh[:64, cc, 2 * cc] = w_d
        wd_sh[64:, cc, 2 * cc + 1] = w_d
        ones_sh[:64, cc, 64 + 2 * cc] = 1
        ones_sh[64:, cc, 64 + 2 * cc + 1] = 1
    wd_sh = wd_sh.reshape(128, 1024)
    ones_sh = ones_sh.reshape(128, 1024)
    W0cat2 = np.zeros((4, 128), f)
    W0cat2[:2, :64] = emb_W; W0cat2[2:, 64:] = emb_W
    embb_cat = np.concatenate([emb_b, emb_b])[:, None].astype(f)
    dbias_sh = np.zeros((128, 1), f)
    dbias_sh[:16] = delta
    return {
        "W1cat": W1cat, "rank1": rank1, "b1_eff": b1_eff, "W2blk": W2blk,
        "b2cat": b2cat, "wd_sh": wd_sh, "ones_sh": ones_sh,
        "W0cat2": W0cat2, "embb_cat": embb_cat, "dbias_sh": dbias_sh,
    }


def _build_nc():
    import concourse.bacc as bacc
    import concourse.mybir as mybir
    import concourse.tile as tile
    from contextlib import ExitStack

    dt = mybir.dt
    f32, f32r = dt.float32, dt.float32r
    AF = mybir.ActivationFunctionType

    nc = bacc.Bacc("TRN2", target_bir_lowering=False, debug=False,
                   num_devices=NCORES)

    din = {}
    MM_INPUTS = {"y_rhs", "vxor", "W1cat", "rank1", "W2blk", "wd_sh",
                 "ones_sh", "W0cat2"}
    for name, shape in [
        ("y_rhs", [4, P]), ("vxor", [10, P]), ("sign_in", [128, S * 256]),
        ("W1cat", [128, 128]), ("rank1", [1, 128]), ("b1_eff", [128, 1]),
        ("W2blk", [128, 128]), ("b2cat", [128, 1]), ("wd_sh", [128, 1024]),
        ("ones_sh", [128, 1024]), ("W0cat2", [4, 128]), ("embb_cat", [128, 1]),
        ("dbias_sh", [128, 1]),
    ]:
        dtt = f32r if name in MM_INPUTS else f32
        din[name] = nc.dram_tensor(name, shape, dtt, kind="ExternalInput").ap()
    dout = {}
    for name in ["loss_o", "p0_o", "p1_o", "nrm_o"]:
        dout[name] = nc.dram_tensor(name, [128, S * 256], f32,
                                    kind="ExternalOutput").ap()
    dall_d = nc.dram_tensor("dall_i", [128, S * 256], f32).ap()
    nall_d = nc.dram_tensor("nall_i", [128, S * 256], f32).ap()

    with tile.TileContext(nc) as tc, ExitStack() as ctx:
        wp = ctx.enter_context(tc.tile_pool(name="w", bufs=1))
        big = ctx.enter_context(tc.tile_pool(name="big", bufs=1))
        yp = ctx.enter_context(tc.tile_pool(name="yp", bufs=3))
        vp = ctx.enter_context(tc.tile_pool(name="vp", bufs=3))
        hp = ctx.enter_context(tc.tile_pool(name="hp", bufs=3))
        sqp = ctx.enter_context(tc.tile_pool(name="sqp", bufs=2))
        esp = ctx.enter_context(tc.tile_pool(name="esp", bufs=2))
        smp = ctx.enter_context(tc.tile_pool(name="smp", bufs=2))
        scp = ctx.enter_context(tc.tile_pool(name="scp", bufs=2))
        fpool = ctx.enter_context(tc.tile_pool(name="fp", bufs=1))
        ph = ctx.enter_context(tc.tile_pool(name="ph", bufs=2, space="PSUM"))
        pe = ctx.enter_context(tc.tile_pool(name="pe", bufs=2, space="PSUM"))
        psm = ctx.enter_context(tc.tile_pool(name="psm", bufs=2, space="PSUM"))

        wt = {}
        for name in ["W1cat", "rank1", "b1_eff", "W2blk", "b2cat", "wd_sh",
                     "ones_sh", "W0cat2", "embb_cat", "dbias_sh"]:
            shape = list(din[name].shape)
            wt[name] = wp.tile(shape, din[name].dtype, name=name, tag=name)
            nc.sync.dma_start(wt[name][:], din[name][:])

        RHS_A = big.tile([128, P], f32r)
        RHS_B = big.tile([128, P], f32r)

        def smalls(psm_t, esv, c):
            cc = c % 8
            nc.tensor.matmul(psm_t[:, :],
                             wt["wd_sh"][:, 128 * cc:128 * (cc + 1)],
                             esv,
                             start=(cc == 0), stop=False,
                             skip_group_check=True)
            sq_t = sqp.tile([128, 512], f32r)
            nc.gpsimd.tensor_mul(sq_t[:], esv, esv)
            nc.tensor.matmul(psm_t[:, :],
                             wt["ones_sh"][:, 128 * cc:128 * (cc + 1)],
                             sq_t[:],
                             start=False, stop=(cc == 7),
                             skip_group_check=True)

        def smalls_out(s, k, psm_t):
            sm_t = smp.tile([128, 512], f32)
            nc.vector.tensor_scalar_add(sm_t[:], psm_t[:], wt["dbias_sh"][:])
            for half, dst in ((0, dall_d), (64, nall_d)):
                src = sm_t[half:half + 16, :].rearrange(
                    "p (qh ql) -> p qh ql", qh=2, ql=256)
                dv = dst.rearrange("(qh k2 ccr) c -> k2 ccr qh c",
                                   qh=2, k2=4, ccr=16)
                nc.sync.dma_start(
                    out=dv[k, :, :, s * 256:(s + 1) * 256], in_=src)

        def q_moves(s, es_t, nxt, g2):
            ql = 1 << (s - 1)
            bqh = 1024 // (2 * ql)
            sc_t = scp.tile([128, 1024], f32r)
            # contiguous partition shifts (DMA can't do strided scatter)
            nc.sync.dma_start(sc_t[0:64, :], es_t[64:128, :])
            nc.sync.dma_start(sc_t[64:128, :], es_t[0:64, :])
            esr = es_t[:].rearrange("p (bqh rest) -> p bqh rest", bqh=bqh)
            scr = sc_t[:].rearrange("p (bqh rest) -> p bqh rest", bqh=bqh)
            dsl = nxt[:, g2 * 1024:(g2 + 1) * 1024].rearrange(
                "p (bqh ql2 r2) -> p bqh ql2 r2", bqh=bqh, ql2=ql, r2=2)
            dsq = dsl[:, :, :, 0]
            dsr = dsl[:, :, :, 1]
            nc.scalar.copy(dsq[0:64], esr[0:64, :, 0:ql])          # Q1
            nc.vector.tensor_copy(dsr[64:128], esr[64:128, :, ql:2 * ql])  # Q4
            nc.gpsimd.tensor_copy(dsr[0:64], scr[0:64, :, 0:ql])   # Q2
            nc.scalar.copy(dsq[64:128], scr[64:128, :, ql:2 * ql])  # Q3

        # ---------------- stage 0 ----------------
        for k in range(4):
            psm_t = psm.tile([128, 512], f32)
            for cc in range(8):
                c = 8 * k + cc
                yt = yp.tile([4, 512], f32r)
                nc.sync.dma_start(yt[:], din["y_rhs"][:, c * 512:(c + 1) * 512])
                ph_t = ph.tile([128, 512], f32)
                nc.tensor.matmul(ph_t[:], wt["W0cat2"][:],
                                 yt[:], start=True, stop=True)
                esv = RHS_A[:, c * 512:(c + 1) * 512]
                nc.vector.tensor_scalar_add(esv, ph_t[:], wt["embb_cat"][:])
                smalls(psm_t, esv, c)
            smalls_out(0, k, psm_t)

        # ---------------- stages 1..10 ----------------
        for s in range(1, 11):
            cur = RHS_A if s % 2 == 1 else RHS_B
            nxt = RHS_B if s % 2 == 1 else RHS_A
            for k in range(4):
                psm_t = psm.tile([128, 512], f32)
                for cc in range(8):
                    c = 8 * k + cc
                    if cc % 2 == 0:
                        es_t = esp.tile([128, 1024], f32r)
                    vx_t = vp.tile([1, 512], f32r)
                    nc.sync.dma_start(
                        vx_t[:], din["vxor"][s - 1:s, c * 512:(c + 1) * 512])
                    ph_t = ph.tile([128, 512], f32)
                    nc.tensor.matmul(ph_t[:], wt["W1cat"][:],
                                     cur[:, c * 512:(c + 1) * 512],
                                     start=True, stop=False,
                                     skip_group_check=True)
                    nc.tensor.matmul(ph_t[:, :],
                                     wt["rank1"][:],
                                     vx_t[:],
                                     start=False, stop=True,
                                     skip_group_check=True)
                    h_t = hp.tile([128, 512], f32r)
                    nc.scalar.activation(h_t[:], ph_t[:], AF.Relu,
                                         bias=wt["b1_eff"][:])
                    pe_t = pe.tile([128, 512], f32)
                    nc.tensor.matmul(pe_t[:], wt["W2blk"][:],
                                     h_t[:], start=True,
                                     stop=True)
                    esv = es_t[:, (cc % 2) * 512:(cc % 2) * 512 + 512]
                    nc.vector.tensor_scalar_add(esv, pe_t[:], wt["b2cat"][:])
                    smalls(psm_t, esv, c)
                    if s < 10 and cc % 2 == 1:
                        q_moves(s, es_t, nxt, c // 2)
                smalls_out(s, k, psm_t)

        # ---------------- final ----------------
        FW = S * 256
        NSL = 8
        SL = FW // NSL                     # 352
        for i in range(NSL):               # loss pass (softplus)
            csl = slice(i * SL, (i + 1) * SL)
            dq = fpool.tile([128, SL], f32, tag="dq")
            nc.sync.dma_start(dq[:], dall_d[:, csl])
            sq = fpool.tile([128, SL], f32, tag="sq")
            nc.sync.dma_start(sq[:], din["sign_in"][:, csl])
            zn = fpool.tile([128, SL], f32, tag="zn")
            nc.vector.tensor_mul(zn[:], sq[:], dq[:])
            ex = fpool.tile([128, SL], f32, tag="ex")
            nc.scalar.activation(ex[:], zn[:], AF.Exp)
            e1 = fpool.tile([128, SL], f32, tag="e1")
            nc.vector.tensor_scalar_add(e1[:], ex[:], 1.0)
            lp = fpool.tile([128, SL], f32, tag="lp")
            nc.scalar.activation(lp[:], e1[:], AF.Ln)
            lo = fpool.tile([128, SL], f32, tag="lo")
            nc.vector.tensor_scalar_min(lo[:], lp[:], LOSS_CAP)
            nc.sync.dma_start(dout["loss_o"][:, csl], lo[:])
        for i in range(NSL):               # pred pass (sigmoid)
            csl = slice(i * SL, (i + 1) * SL)
            dq = fpool.tile([128, SL], f32, tag="dq")
            nc.sync.dma_start(dq[:], dall_d[:, csl])
            p1t = fpool.tile([128, SL], f32, tag="p1")
            nc.scalar.activation(p1t[:], dq[:], AF.Sigmoid)
            nc.sync.dma_start(dout["p1_o"][:, csl], p1t[:])
            p0t = fpool.tile([128, SL], f32, tag="p0")
            nc.scalar.activation(p0t[:], dq[:], AF.Sigmoid, scale=-1.0)
            nc.sync.dma_start(dout["p0_o"][:, csl], p0t[:])
        for i in range(NSL):               # norm pass (sqrt)
            csl = slice(i * SL, (i + 1) * SL)
            nq = fpool.tile([128, SL], f32, tag="nq")
            nc.sync.dma_start(nq[:], nall_d[:, csl])
            nrt = fpool.tile([128, SL], f32, tag="nr")
            nc.scalar.activation(nrt[:], nq[:], AF.Sqrt)
            nc.sync.dma_start(dout["nrm_o"][:, csl], nrt[:])

    nc.compile()
    return nc


def _unpack(core_outs):
    """core_outs: dict of [128, S*256] arrays -> (losses, preds, norms)."""
    b = BLOC

    def unpk(a):
        a = a.reshape(2, 64, S, 256)            # (qh, 2b+r, s, ql)
        a = a.reshape(2, b, 2, S, 256)          # (qh, bb, r, s, ql)
        a = a.transpose(3, 2, 1, 0, 4).reshape(S, 2, b, 512)
        return a                                # [s, r, b, q]

    lo = unpk(core_outs["loss_o"]); p0 = unpk(core_outs["p0_o"])
    p1 = unpk(core_outs["p1_o"]); nr = unpk(core_outs["nrm_o"])
    losses = np.zeros((b, S, N), np.float32)
    preds = np.zeros((b, N, S, 2), np.float32)
    norms = np.zeros((b, S, N), np.float32)
    for r in range(2):
        losses[:, :, r::2] = lo[:, r].transpose(1, 0, 2)
        norms[:, :, r::2] = nr[:, r].transpose(1, 0, 2)
        preds[:, r::2, :, 0] = p0[:, r].transpose(1, 2, 0)
        preds[:, r::2, :, 1] = p1[:, r].transpose(1, 2, 0)
    return losses, preds, norms


def kernel(x, y, emb_W, emb_b, lab_emb, cn_W1, cn_b1, cn_W2, cn_b2,
           bn_W1, bn_b1, bn_W2, bn_b2, llr_W, llr_b, _trace=False):
    from concourse.bass_utils import run_bass_kernel_spmd

    inp = dict(x=np.asarray(x), y=np.asarray(y))
    w = _fold_weights(dict(
        emb_W=np.asarray(emb_W), emb_b=np.asarray(emb_b),
        lab_emb=np.asarray(lab_emb), cn_W1=np.asarray(cn_W1),
        cn_b1=np.asarray(cn_b1), cn_W2=np.asarray(cn_W2),
        cn_b2=np.asarray(cn_b2), bn_W1=np.asarray(bn_W1),
        bn_b1=np.asarray(bn_b1), bn_W2=np.asarray(bn_W2),
        bn_b2=np.asarray(bn_b2), llr_W=np.asarray(llr_W),
        llr_b=np.asarray(llr_b)))

    if "nc" not in _COMPILED:
        _COMPILED["nc"] = _build_nc()
    nc = _COMPILED["nc"]

    in_maps = []
    for ci in range(NCORES):
        sl = slice(ci * BLOC, (ci + 1) * BLOC)
        in_maps.append(_host_prep(inp["y"][sl], inp["x"][sl], w))

    res = run_bass_kernel_spmd(nc, in_maps, list(range(NCORES)),
                               trace=_trace)
    losses = np.zeros((B, S, N), np.float32)
    preds = np.zeros((B, N, S, 2), np.float32)
    norms = np.zeros((B, S, N), np.float32)
    for ci in range(NCORES):
        lo, pr, no = _unpack(res.results[ci])
        sl = slice(ci * BLOC, (ci + 1) * BLOC)
        losses[sl], preds[sl], norms[sl] = lo, pr, no
    kernel._last = res
    return losses, preds, norms


# revision 16
# speedup vs baseline: 1.2344x; 1.0515x over previous
"""Trainium2 Bass kernel for nn_NeuralPolarDecoder.

Data-parallel over 8 NeuronCores (batch 256 -> 32/core). Per core, the
polar-decoder stage recursion runs with features on SBUF partitions and
(sample, pair) on the free axis:

  RHS_s [128, 16384]: rows 0:64 = odd-operand features, 64:128 = even-operand.
  stage: h = relu(W1cat.T @ RHS + vxor-rank1 + b1)   (one K=128 matmul + K=1)
         e' = W2blk.T @ h  (block-diag cn/bn)  -> psum [e'L ; e'R]
         ES = e' + b2cat   (SBUF scratch)
         d  = wd.T @ ES  (pred head), nsq = ones.T @ ES^2 -- via shifted
         weights, 8 chunks accumulate into one psum bank at rows 0:16/64:80
         RHS_{s+1} built from ES by 4 quadrant moves (2 strided engine copies,
         2 partition-crossing DMAs).
  Final batch: softplus/sigmoid/sqrt over stacked per-stage d/nsq planes
  (round-tripped through internal DRAM to fit SBUF).

Label-bit (v) evolution and all weight folding are precomputed on host;
device outputs are packed [128, 11*256] tiles unpacked on host.
"""

import numpy as np

B, N, D = 256, 1024, 64
NCORES = 8
BLOC = B // NCORES          # 32 samples per core
T = BLOC * N                # 32768 tokens per core
P = T // 2                  # 16384 pairs per core
S = 11                      # stages (incl. stage 0)
LOSS_CAP = 16.11809565095832

_COMPILED = {}


def _host_prep(y, x, w):
    """Per-core host packing. y (32,1024,2) f32, x (32,1024,1) i32."""
    b = y.shape[0]
    yf = y.reshape(b * N, 2)
    y_rhs = np.concatenate([yf[0::2].T, yf[1::2].T], axis=0).astype(np.float32)  # [4,P]

    vflat = x.reshape(b * N).astype(np.float32)
    Vo = vflat[0::2].copy()
    Ve = vflat[1::2].copy()
    vxor_rows = np.zeros((10, P), np.float32)
    sign_in = np.zeros((128, S * 256), np.float32)

    def pack_sign(s, vL, vR):
        # value (r,bb,q) -> [p = (q//256)*64 + 2bb + r, col s*256 + q%256]
        for r, v in ((0, vL), (1, vR)):
            vv = v.reshape(b, 512)
            for qh in range(2):
                blk = 1.0 - 2.0 * vv[:, qh * 256:(qh + 1) * 256]
                sign_in[qh * 64 + 2 * np.arange(b) + r, s * 256:(s + 1) * 256] = blk

    pack_sign(0, Vo, Ve)
    q = np.arange(512)
    for s in range(1, 11):
        vx = (Vo - Ve) ** 2
        vxor_rows[s - 1] = vx
        pack_sign(s, vx, Ve)
        if s < 10:
            bit = (q >> (s - 1)) & 1
            qdel = ((q >> s) << (s - 1)) | (q & ((1 << (s - 1)) - 1))
            m0 = bit == 0
            jp0 = 2 * qdel[m0]
            jp1 = 2 * qdel[~m0]
            Vor = Vo.reshape(b, 512); Ver = Ve.reshape(b, 512)
            vxr = vx.reshape(b, 512)
            nVo = np.empty_like(Vor); nVe = np.empty_like(Ver)
            nVo[:, jp0] = vxr[:, q[m0]]; nVo[:, jp0 + 1] = Ver[:, q[m0]]
            nVe[:, jp1] = vxr[:, q[~m0]]; nVe[:, jp1 + 1] = Ver[:, q[~m0]]
            Vo, Ve = nVo.reshape(P), nVe.reshape(P)

    return {"y_rhs": y_rhs, "vxor": vxor_rows, "sign_in": sign_in, **w}


def _fold_weights(inp):
    f = np.float32
    cn_W1, cn_b1 = inp["cn_W1"].astype(f), inp["cn_b1"].astype(f)
    cn_W2, cn_b2 = inp["cn_W2"].astype(f), inp["cn_b2"].astype(f)
    bn_W1, bn_b1 = inp["bn_W1"].astype(f), inp["bn_b1"].astype(f)
    bn_W2, bn_b2 = inp["bn_W2"].astype(f), inp["bn_b2"].astype(f)
    emb_W, emb_b = inp["emb_W"].astype(f), inp["emb_b"].astype(f)
    lab_emb = inp["lab_emb"].astype(f)
    llr_W, llr_b = inp["llr_W"].astype(f), inp["llr_b"].astype(f)

    W1cat = np.concatenate([cn_W1, bn_W1[:128]], axis=1)            # [128,128]
    r0 = lab_emb[0] @ bn_W1[128:192]
    r1 = lab_emb[1] @ bn_W1[128:192]
    rank1 = np.concatenate([np.zeros(64, f), (r1 - r0).astype(f)])[None, :]  # [1,128]
    b1_eff = np.concatenate([cn_b1, bn_b1 + r0])[:, None].astype(f)
    W2blk = np.zeros((128, 128), f)
    W2blk[:64, :64] = cn_W2; W2blk[64:, 64:] = bn_W2
    b2cat = np.concatenate([cn_b2, bn_b2])[:, None].astype(f)
    w_d = (llr_W[:, 1] - llr_W[:, 0]).astype(f)
    delta = float(llr_b[1] - llr_b[0])
    # shifted weights: block cc has the head at cols 2cc:2cc+2 of [128,16]
    wd_sh = np.zeros((128, 8, 128), f)
    ones_sh = np.zeros((128, 8, 128), f)
    for cc in range(8):
        wd_sh[:64, cc, 2 * cc] = w_d
        wd_sh[64:, cc, 2 * cc + 1] = w_d
        ones_sh[:64, cc, 64 + 2 * cc] = 1
        ones_sh[64:, cc, 64 + 2 * cc + 1] = 1
    wd_sh = wd_sh.reshape(128, 1024)
    ones_sh = ones_sh.reshape(128, 1024)
    W0cat2 = np.zeros((4, 128), f)
    W0cat2[:2, :64] = emb_W; W0cat2[2:, 64:] = emb_W
    embb_cat = np.concatenate([emb_b, emb_b])[:, None].astype(f)
    dbias_sh = np.zeros((128, 1), f)
    dbias_sh[:16] = delta
    return {
        "W1cat": W1cat, "rank1": rank1, "b1_eff": b1_eff, "W2blk": W2blk,
        "b2cat": b2cat, "wd_sh": wd_sh, "ones_sh": ones_sh,
        "W0cat2": W0cat2, "embb_cat": embb_cat, "dbias_sh": dbias_sh,
    }


def _build_nc():
    import concourse.bacc as bacc
    import concourse.mybir as mybir
    import concourse.tile as tile
    from contextlib import ExitStack

    dt = mybir.dt
    f32, f32r = dt.float32, dt.float32r
    AF = mybir.ActivationFunctionType

    nc = bacc.Bacc("TRN2", target_bir_lowering=False, debug=False,
                   num_devices=NCORES)

    din = {}
    MM_INPUTS = {"y_rhs", "vxor", "W1cat", "rank1", "W2blk", "wd_sh",
                 "ones_sh", "W0cat2"}
    for name, shape in [
        ("y_rhs", [4, P]), ("vxor", [10, P]), ("sign_in", [128, S * 256]),
        ("W1cat", [128, 128]), ("rank1", [1, 128]), ("b1_eff", [128, 1]),
        ("W2blk", [128, 128]), ("b2cat", [128, 1]), ("wd_sh", [128, 1024]),
        ("ones_sh", [128, 1024]), ("W0cat2", [4, 128]), ("embb_cat", [128, 1]),
        ("dbias_sh", [128, 1]),
    ]:
        dtt = f32r if name in MM_INPUTS else f32
        din[name] = nc.dram_tensor(name, shape, dtt, kind="ExternalInput").ap()
    dout = {}
    for name in ["loss_o", "p0_o", "p1_o", "nrm_o"]:
        dout[name] = nc.dram_tensor(name, [128, S * 256], f32,
                                    kind="ExternalOutput").ap()
    dall_d = nc.dram_tensor("dall_i", [128, S * 256], f32).ap()
    nall_d = nc.dram_tensor("nall_i", [128, S * 256], f32).ap()

    with tile.TileContext(nc) as tc, ExitStack() as ctx:
        wp = ctx.enter_context(tc.tile_pool(name="w", bufs=1))
        big = ctx.enter_context(tc.tile_pool(name="big", bufs=1))
        yp = ctx.enter_context(tc.tile_pool(name="yp", bufs=3))
        vp = ctx.enter_context(tc.tile_pool(name="vp", bufs=3))
        hp = ctx.enter_context(tc.tile_pool(name="hp", bufs=3))
        sqp = ctx.enter_context(tc.tile_pool(name="sqp", bufs=2))
        esp = ctx.enter_context(tc.tile_pool(name="esp", bufs=2))
        smp = ctx.enter_context(tc.tile_pool(name="smp", bufs=2))
        scp = ctx.enter_context(tc.tile_pool(name="scp", bufs=2))
        fpool = ctx.enter_context(tc.tile_pool(name="fp", bufs=1))
        ph = ctx.enter_context(tc.tile_pool(name="ph", bufs=2, space="PSUM"))
        pe = ctx.enter_context(tc.tile_pool(name="pe", bufs=2, space="PSUM"))
        psm = ctx.enter_context(tc.tile_pool(name="psm", bufs=2, space="PSUM"))

        wt = {}
        for name in ["W1cat", "rank1", "b1_eff", "W2blk", "b2cat", "wd_sh",
                     "ones_sh", "W0cat2", "embb_cat", "dbias_sh"]:
            shape = list(din[name].shape)
            wt[name] = wp.tile(shape, din[name].dtype, name=name, tag=name)
            nc.sync.dma_start(wt[name][:], din[name][:])

        RHS_A = big.tile([128, P], f32r)
        RHS_B = big.tile([128, P], f32r)

        def smalls(psm_t, esv, c):
            cc = c % 8
            nc.tensor.matmul(psm_t[:, :],
                             wt["wd_sh"][:, 128 * cc:128 * (cc + 1)],
                             esv,
                             start=(cc == 0), stop=False,
                             skip_group_check=True)
            sq_t = sqp.tile([128, 512], f32r)
            nc.gpsimd.tensor_mul(sq_t[:], esv, esv)
            nc.tensor.matmul(psm_t[:, :],
                             wt["ones_sh"][:, 128 * cc:128 * (cc + 1)],
                             sq_t[:],
                             start=False, stop=(cc == 7),
                             skip_group_check=True)

        def smalls_out(s, k, psm_t):
            sm_t = smp.tile([128, 512], f32)
            nc.vector.tensor_scalar_add(sm_t[:], psm_t[:], wt["dbias_sh"][:])
            for half, dst in ((0, dall_d), (64, nall_d)):
                src = sm_t[half:half + 16, :].rearrange(
                    "p (qh ql) -> p qh ql", qh=2, ql=256)
                dv = dst.rearrange("(qh k2 ccr) c -> k2 ccr qh c",
                                   qh=2, k2=4, ccr=16)
                nc.sync.dma_start(
                    out=dv[k, :, :, s * 256:(s + 1) * 256], in_=src)

        def q_moves(s, es_t, nxt, g2):
            ql = 1 << (s - 1)
            bqh = 1024 // (2 * ql)
            sc_t = scp.tile([128, 1024], f32r)
            # contiguous partition shifts (DMA can't do strided scatter)
            nc.sync.dma_start(sc_t[0:64, :], es_t[64:128, :])
            nc.sync.dma_start(sc_t[64:128, :], es_t[0:64, :])
            esr = es_t[:].rearrange("p (bqh rest) -> p bqh rest", bqh=bqh)
            scr = sc_t[:].rearrange("p (bqh rest) -> p bqh rest", bqh=bqh)
            dsl = nxt[:, g2 * 1024:(g2 + 1) * 1024].rearrange(
                "p (bqh ql2 r2) -> p bqh ql2 r2", bqh=bqh, ql2=ql, r2=2)
            dsq = dsl[:, :, :, 0]
            dsr = dsl[:, :, :, 1]
            nc.scalar.copy(dsq[0:64], esr[0:64, :, 0:ql])          # Q1
            nc.vector.tensor_copy(dsr[64:128], esr[64:128, :, ql:2 * ql])  # Q4
            nc.gpsimd.tensor_copy(dsr[0:64], scr[0:64, :, 0:ql])   # Q2
            nc.vector.tensor_copy(dsq[64:128], scr[64:128, :, ql:2 * ql])  # Q3

        # ---------------- stage 0 ----------------
        for k in range(4):
            psm_t = psm.tile([128, 512], f32)
            for cc in range(8):
                c = 8 * k + cc
                yt = yp.tile([4, 512], f32r)
                nc.sync.dma_start(yt[:], din["y_rhs"][:, c * 512:(c + 1) * 512])
                ph_t = ph.tile([128, 512], f32)
                nc.tensor.matmul(ph_t[:], wt["W0cat2"][:],
                                 yt[:], start=True, stop=True)
                esv = RHS_A[:, c * 512:(c + 1) * 512]
                nc.vector.tensor_scalar_add(esv, ph_t[:], wt["embb_cat"][:])
                smalls(psm_t, esv, c)
            smalls_out(0, k, psm_t)

        # ---------------- stages 1..10 ----------------
        for s in range(1, 11):
            cur = RHS_A if s % 2 == 1 else RHS_B
            nxt = RHS_B if s % 2 == 1 else RHS_A
            for k in range(4):
                psm_t = psm.tile([128, 512], f32)
                for cc in range(8):
                    c = 8 * k + cc
                    if cc % 2 == 0:
                        es_t = esp.tile([128, 1024], f32r)
                    vx_t = vp.tile([1, 512], f32r)
                    nc.sync.dma_start(
                        vx_t[:], din["vxor"][s - 1:s, c * 512:(c + 1) * 512])
                    ph_t = ph.tile([128, 512], f32)
                    nc.tensor.matmul(ph_t[:], wt["W1cat"][:],
                                     cur[:, c * 512:(c + 1) * 512],
                                     start=True, stop=False,
                                     skip_group_check=True)
                    nc.tensor.matmul(ph_t[:, :],
                                     wt["rank1"][:],
                                     vx_t[:],
                                     start=False, stop=True,
                                     skip_group_check=True)
                    h_t = hp.tile([128, 512], f32r)
                    nc.scalar.activation(h_t[:], ph_t[:], AF.Relu,
                                         bias=wt["b1_eff"][:])
                    pe_t = pe.tile([128, 512], f32)
                    nc.tensor.matmul(pe_t[:], wt["W2blk"][:],
                                     h_t[:], start=True,
                                     stop=True)
                    esv = es_t[:, (cc % 2) * 512:(cc % 2) * 512 + 512]
                    nc.vector.tensor_scalar_add(esv, pe_t[:], wt["b2cat"][:])
                    smalls(psm_t, esv, c)
                    if s < 10 and cc % 2 == 1:
                        q_moves(s, es_t, nxt, c // 2)
                smalls_out(s, k, psm_t)

        # ---------------- final ----------------
        FW = S * 256
        NSL = 8
        SL = FW // NSL                     # 352
        for i in range(NSL):               # loss pass (softplus)
            csl = slice(i * SL, (i + 1) * SL)
            dq = fpool.tile([128, SL], f32, tag="dq")
            nc.sync.dma_start(dq[:], dall_d[:, csl])
            sq = fpool.tile([128, SL], f32, tag="sq")
            nc.sync.dma_start(sq[:], din["sign_in"][:, csl])
            zn = fpool.tile([128, SL], f32, tag="zn")
            nc.vector.tensor_mul(zn[:], sq[:], dq[:])
            ex = fpool.tile([128, SL], f32, tag="ex")
            nc.scalar.activation(ex[:], zn[:], AF.Exp)
            e1 = fpool.tile([128, SL], f32, tag="e1")
            nc.vector.tensor_scalar_add(e1[:], ex[:], 1.0)
            lp = fpool.tile([128, SL], f32, tag="lp")
            nc.scalar.activation(lp[:], e1[:], AF.Ln)
            lo = fpool.tile([128, SL], f32, tag="lo")
            nc.vector.tensor_scalar_min(lo[:], lp[:], LOSS_CAP)
            nc.sync.dma_start(dout["loss_o"][:, csl], lo[:])
        for i in range(NSL):               # pred pass (sigmoid)
            csl = slice(i * SL, (i + 1) * SL)
            dq = fpool.tile([128, SL], f32, tag="dq")
            nc.sync.dma_start(dq[:], dall_d[:, csl])
            p1t = fpool.tile([128, SL], f32, tag="p1")
            nc.scalar.activation(p1t[:], dq[:], AF.Sigmoid)
            nc.sync.dma_start(dout["p1_o"][:, csl], p1t[:])
            p0t = fpool.tile([128, SL], f32, tag="p0")
            nc.scalar.activation(p0t[:], dq[:], AF.Sigmoid, scale=-1.0)
            nc.sync.dma_start(dout["p0_o"][:, csl], p0t[:])
        for i in range(NSL):               # norm pass (sqrt)
            csl = slice(i * SL, (i + 1) * SL)
            nq = fpool.tile([128, SL], f32, tag="nq")
            nc.sync.dma_start(nq[:], nall_d[:, csl])
            nrt = fpool.tile([128, SL], f32, tag="nr")
            nc.scalar.activation(nrt[:], nq[:], AF.Sqrt)
            nc.sync.dma_start(dout["nrm_o"][:, csl], nrt[:])

    nc.compile()
    return nc


def _unpack(core_outs):
    """core_outs: dict of [128, S*256] arrays -> (losses, preds, norms)."""
    b = BLOC

    def unpk(a):
        a = a.reshape(2, 64, S, 256)            # (qh, 2b+r, s, ql)
        a = a.reshape(2, b, 2, S, 256)          # (qh, bb, r, s, ql)
        a = a.transpose(3, 2, 1, 0, 4).reshape(S, 2, b, 512)
        return a                                # [s, r, b, q]

    lo = unpk(core_outs["loss_o"]); p0 = unpk(core_outs["p0_o"])
    p1 = unpk(core_outs["p1_o"]); nr = unpk(core_outs["nrm_o"])
    losses = np.zeros((b, S, N), np.float32)
    preds = np.zeros((b, N, S, 2), np.float32)
    norms = np.zeros((b, S, N), np.float32)
    for r in range(2):
        losses[:, :, r::2] = lo[:, r].transpose(1, 0, 2)
        norms[:, :, r::2] = nr[:, r].transpose(1, 0, 2)
        preds[:, r::2, :, 0] = p0[:, r].transpose(1, 2, 0)
        preds[:, r::2, :, 1] = p1[:, r].transpose(1, 2, 0)
    return losses, preds, norms


def kernel(x, y, emb_W, emb_b, lab_emb, cn_W1, cn_b1, cn_W2, cn_b2,
           bn_W1, bn_b1, bn_W2, bn_b2, llr_W, llr_b, _trace=False):
    from concourse.bass_utils import run_bass_kernel_spmd

    inp = dict(x=np.asarray(x), y=np.asarray(y))
    w = _fold_weights(dict(
        emb_W=np.asarray(emb_W), emb_b=np.asarray(emb_b),
        lab_emb=np.asarray(lab_emb), cn_W1=np.asarray(cn_W1),
        cn_b1=np.asarray(cn_b1), cn_W2=np.asarray(cn_W2),
        cn_b2=np.asarray(cn_b2), bn_W1=np.asarray(bn_W1),
        bn_b1=np.asarray(bn_b1), bn_W2=np.asarray(bn_W2),
        bn_b2=np.asarray(bn_b2), llr_W=np.asarray(llr_W),
        llr_b=np.asarray(llr_b)))

    if "nc" not in _COMPILED:
        _COMPILED["nc"] = _build_nc()
    nc = _COMPILED["nc"]

    in_maps = []
    for ci in range(NCORES):
        sl = slice(ci * BLOC, (ci + 1) * BLOC)
        in_maps.append(_host_prep(inp["y"][sl], inp["x"][sl], w))

    res = run_bass_kernel_spmd(nc, in_maps, list(range(NCORES)),
                               trace=_trace)
    losses = np.zeros((B, S, N), np.float32)
    preds = np.zeros((B, N, S, 2), np.float32)
    norms = np.zeros((B, S, N), np.float32)
    for ci in range(NCORES):
        lo, pr, no = _unpack(res.results[ci])
        sl = slice(ci * BLOC, (ci + 1) * BLOC)
        losses[sl], preds[sl], norms[sl] = lo, pr, no
    kernel._last = res
    return losses, preds, norms
